# revision 3
# baseline (speedup 1.0000x reference)
"""AttentionConv (7x7 windowed per-channel softmax attention) on 8 TRN2 cores.

V2: d1-grouped mega-ops.  Sharding: core = (chalf, batch, shalf); chalf=1
maps stored transposed so rel_w folds like rel_h.  Per core: 128 channels
x 28x56 positions.

Per-rep structure (7 d1-iterations instead of 49 offset-iterations):
  phase 1 (PE, fp16 weights/x): q/k/v projections -> fp32 PSUM;
    k -> fp16 kpad, q -> fp16 qsb, v -> fp16 vpad (cast copies).
  per d1:
    kb = kpad rows + rel[:,d1]        one DVE tensor_scalar (4x fp16)
    s  = kb_view * q                  ONE DVE TT (2x fp16), 3-D in0 AP
                                      [d2:1, r:62, w:1], q bcast over d2
    e  = exp(s - 48) -> bf16          ONE ACT instr (10976 cols)
    t  = e * v_view -> bf16           ONE TT (DVE or Pool per T_POOL)
    den/num: per d2, per col-slice: PE identity matmuls accumulate into
      3x512 den + 3x512 num PSUM banks + shared 32-col tails (start=False
      on pre-zeroed sub-banks)
  out = num * reciprocal(den)         DVE
Cross-rep pipelining: next rep's phase-1 items drain between (d1,d2) MM
slices; PSUM layout: den 3 + num 3 + (tails+mm2) 1 + mm 1 = 8 banks, proj
groups double-buffered across mm/mm2.
"""
import numpy as np
from contextlib import ExitStack

import jax
from jax.sharding import Mesh, PartitionSpec
from jax.experimental.shard_map import shard_map

import concourse.bass as bass
import concourse.bacc as bacc
import concourse.tile as tile
from concourse import mybir
from concourse import bass2jax

F32 = mybir.dt.float32
BF16 = mybir.dt.bfloat16
FP16 = mybir.dt.float16

B, H, W, CIN, CO, K, PAD = 2, 56, 56, 512, 256, 7, 3
OWN = 28
SPAN = 31
PR = 34
PW = 62
NPOS = PR * 56      # 1904
NOWN = OWN * 56     # 1568
SHIFT = -48.0
NSL = 4
SLW = NOWN // NSL   # 392
ND1 = NOWN * K      # 10976 cols per d1 group

_CACHE = {}
# per d1 group, the t-mult for d2 in [0, t_pool[d1]) runs on GpSimd(Pool),
# the rest on DVE
T_POOL = (2, 2, 2, 2, 2, 2, 2)
BUFS = 2
NCH_DMA = 8
V_COPY = "act"      # engine for v PSUM->SBUF cast copies
K_COPY = "act"      # engine for k PSUM->SBUF cast copies
FIN_MULT = "dve"    # engine for the final num*recip multiplies
GEXP = 2            # exp instructions per d1 group


def _build_nc(reps=1, t_pool=T_POOL, bufs=BUFS, nch=NCH_DMA,
              v_copy=V_COPY, k_copy=K_COPY, fin_mult=FIN_MULT, gexp=GEXP):
    nc = bacc.Bacc("TRN2", target_bir_lowering=False, debug=False)
    xt = nc.dram_tensor("xt", [CIN, NPOS], FP16, kind="ExternalInput").ap()
    wt = nc.dram_tensor("wt", [3, CIN, 128], FP16, kind="ExternalInput").ap()
    rel = nc.dram_tensor("rel", [128, K], F32, kind="ExternalInput").ap()
    ident = nc.dram_tensor("ident", [128, 128], BF16, kind="ExternalInput").ap()
    nbias = nc.dram_tensor("nbias", [128, 1], F32, kind="ExternalInput").ap()
    out = nc.dram_tensor("out", [128, NOWN], F32, kind="ExternalOutput").ap()

    # den/num col splits: 3x512 + 32-col tails in the shared dnt bank
    slc = [(0, 512), (512, 512), (1024, 512)]

    with tile.TileContext(nc) as tc, ExitStack() as ctx:
        per = ctx.enter_context(tc.tile_pool(name="per", bufs=1))
        ld = ctx.enter_context(tc.tile_pool(name="ld", bufs=1))

        wsb = ld.tile([128, 3, 4, 128], FP16)
        wtv = wt.rearrange("w (t p) m -> p w t m", p=128)
        nc.sync.dma_start(out=wsb[:, 1], in_=wtv[:, 1])   # k weights
        nc.sync.dma_start(out=wsb[:, 0], in_=wtv[:, 0])   # q weights
        relsb = per.tile([128, K], F32)
        nc.sync.dma_start(out=relsb, in_=rel)
        identsb = per.tile([128, 128], BF16)
        nc.sync.dma_start(out=identsb, in_=ident)
        nbsb = per.tile([128, 1], F32)
        nc.sync.dma_start(out=nbsb, in_=nbias)
        xsb = ld.tile([128, 4, NPOS], FP16)
        xtv = xt.rearrange("(t p) n -> p t n", p=128)
        chw = NPOS // nch
        for c in range(nch):
            for t in range(4):
                nc.sync.dma_start(out=xsb[:, t, c * chw:(c + 1) * chw],
                                  in_=xtv[:, t, c * chw:(c + 1) * chw])
        nc.sync.dma_start(out=wsb[:, 2], in_=wtv[:, 2])   # v weights

        maps = ctx.enter_context(tc.tile_pool(name="maps", bufs=2))
        kbp = ctx.enter_context(tc.tile_pool(name="kbp", bufs=3))
        sp = ctx.enter_context(tc.tile_pool(name="sp", bufs=bufs))
        ep = ctx.enter_context(tc.tile_pool(name="ep", bufs=bufs))
        tp = ctx.enter_context(tc.tile_pool(name="tp", bufs=bufs))
        fin = ctx.enter_context(tc.tile_pool(name="fin", bufs=2))
        # static PSUM layout (8 banks): mm 1 + den 3 + num 3 + (dnt+mm2) 1
        mm = ctx.enter_context(tc.tile_pool(name="mm", bufs=1, space="PSUM"))
        accp = ctx.enter_context(tc.tile_pool(name="acc", bufs=1,
                                              space="PSUM"))

        kv_slices = [(0, 7), (7, 7), (14, 7), (21, 7), (28, 6)]

        def make_p1(ri):
            """Allocate rep ri's map tiles and return (tiles, emit-closures)."""
            kpad = maps.tile([128, PR, PW], FP16, tag="kpad", name=f"kpad{ri}")
            vpad = maps.tile([128, PR, PW], FP16, tag="vpad", name=f"vpad{ri}")
            qsb = maps.tile([128, NOWN], FP16, tag="qsb", name=f"qsb{ri}")
            tiles = (kpad, vpad, qsb)

            def memsets():
                for buf in (kpad, vpad):
                    nc.gpsimd.memset(buf[:, :, 0:PAD], 0.0)
                    nc.gpsimd.memset(buf[:, :, PAD + 56:PW], 0.0)

            def proj_kv(wi, dst, r0, nr, eng_copy):
                pt = mm.tile([128, 392], F32, tag="mmkv", name="pt")
                ptv = pt[:, :nr * 56]
                n0, n1 = r0 * 56, (r0 + nr) * 56
                for t in range(4):
                    nc.tensor.matmul(ptv,
                                     lhsT=wsb[:, wi, t, :],
                                     rhs=xsb[:, t, n0:n1],
                                     start=(t == 0), stop=(t == 3))
                dstv = dst[:, r0:r0 + nr, PAD:PAD + 56]
                srcv = ptv.rearrange("p (r c) -> p r c", r=nr)
                if eng_copy == "act":
                    nc.scalar.copy(out=dstv, in_=srcv)
                elif eng_copy == "pool":
                    nc.gpsimd.tensor_copy(out=dstv, in_=srcv)
                else:
                    nc.vector.tensor_copy(out=dstv, in_=srcv)

            def proj_q(i):
                pt = mm.tile([128, 392], F32, tag="mmkv", name="pt")
                n0 = PAD * 56 + i * SLW
                for t in range(4):
                    nc.tensor.matmul(pt, lhsT=wsb[:, 0, t, :],
                                     rhs=xsb[:, t, n0:n0 + SLW],
                                     start=(t == 0), stop=(t == 3))
                nc.scalar.copy(out=qsb[:, i * SLW:(i + 1) * SLW], in_=pt)

            from functools import partial
            items = [memsets]
            items += [partial(proj_kv, 1, kpad, r0, nr, k_copy)
                      for (r0, nr) in kv_slices]
            items += [partial(proj_q, i) for i in range(NSL)]
            items += [partial(proj_kv, 2, vpad, r0, nr, v_copy)
                      for (r0, nr) in kv_slices]
            return tiles, items

        def phase2(tiles, nxt_items):
            """Emit rep's attention stream, draining nxt_items between MM
            slices."""
            kpad, vpad, qsb = tiles
            den = accp.tile([128, 3, 512], F32, tag="den", name="den")
            num = accp.tile([128, 3, 512], F32, tag="num", name="num")
            dnt = accp.tile([128, 2, 32], F32, tag="dnt", name="dnt")
            nc.vector.memset(dnt, 0.0)

            q3 = qsb.rearrange("p (r c) -> p r c", r=OWN)
            nit = len(nxt_items)
            drained = 0
            j = 0
            NSLOT = K * K
            for d1 in range(K):
                kb = kbp.tile([128, OWN, PW], FP16, tag="kb", name=f"kb{d1}")
                nc.vector.tensor_scalar_add(out=kb,
                                            in0=kpad[:, d1:d1 + OWN, :],
                                            scalar1=relsb[:, d1:d1 + 1])
                st = sp.tile([128, K, OWN, 56], FP16, tag="s", name="st")
                # in0[d2, r, w] = kb[r, d2 + w]; q broadcast over d2
                kbw = bass.AP(tensor=kb.tensor, offset=kb.offset,
                              ap=mybir.VecI64Pair(
                                  [list(kb.ap[0]), [1, K], [PW, OWN],
                                   [1, 56]]))
                qb = bass.AP(tensor=q3.tensor, offset=q3.offset,
                             ap=mybir.VecI64Pair(
                                 [list(q3.ap[0]), [0, K], [56, OWN],
                                  [1, 56]]))
                nc.vector.tensor_tensor(
                    out=st,
                    in0=kbw,
                    in1=qb,
                    op=mybir.AluOpType.mult)
                et = ep.tile([128, K, OWN, 56], BF16, tag="e", name="et")
                esplits = ([(0, K)] if gexp == 1 else
                           [(i * K // gexp, (i + 1) * K // gexp)
                            for i in range(gexp)])
                for lo, hi in esplits:
                    nc.scalar.activation(
                        out=et[:, lo:hi], in_=st[:, lo:hi],
                        func=mybir.ActivationFunctionType.Exp,
                        bias=nbsb, scale=1.0)
                tt = tp.tile([128, K, OWN, 56], BF16, tag="t", name="tt")

                def t_part(eng, lo, hi):
                    vw = bass.AP(
                        tensor=vpad.tensor,
                        offset=vpad.offset + d1 * PW + lo,
                        ap=mybir.VecI64Pair(
                            [list(vpad.ap[0]), [1, hi - lo], [PW, OWN],
                             [1, 56]]))
                    eng.tensor_tensor(out=tt[:, lo:hi], in0=et[:, lo:hi],
                                      in1=vw, op=mybir.AluOpType.mult)

                np_ = t_pool[d1]
                for i in range(np_):           # single-d2 Pool ops
                    t_part(nc.gpsimd, i, i + 1)
                if np_ < K:
                    t_part(nc.vector, np_, K)

                e2 = et.rearrange("p k r c -> p (k r c)")
                t2 = tt.rearrange("p k r c -> p (k r c)")
                for d2 in range(K):
                    want = min(nit, nit * (j + 1) // NSLOT + 1)
                    while drained < want:
                        nxt_items[drained]()
                        drained += 1
                    first = (d1 == 0 and d2 == 0)
                    last = (d1 == K - 1 and d2 == K - 1)
                    o2 = d2 * NOWN
                    for i, (c0, cw) in enumerate(slc):
                        nc.tensor.matmul(
                            den[:, i, :cw], lhsT=identsb,
                            rhs=e2[:, o2 + c0:o2 + c0 + cw],
                            start=first, stop=last, skip_group_check=True)
                        nc.tensor.matmul(
                            num[:, i, :cw], lhsT=identsb,
                            rhs=t2[:, o2 + c0:o2 + c0 + cw],
                            start=first, stop=last, skip_group_check=True)
                    nc.tensor.matmul(
                        dnt[:, 0, :], lhsT=identsb,
                        rhs=e2[:, o2 + 1536:o2 + NOWN],
                        start=False, stop=last, skip_group_check=True)
                    nc.tensor.matmul(
                        dnt[:, 1, :], lhsT=identsb,
                        rhs=t2[:, o2 + 1536:o2 + NOWN],
                        start=False, stop=last, skip_group_check=True)
                    j += 1
            while drained < nit:
                nxt_items[drained]()
                drained += 1

            rden = fin.tile([128, NOWN], F32, tag="rden", name="rden")
            outsb = fin.tile([128, NOWN], F32, tag="outsb", name="outsb")
            views = [(c0, cw, den[:, i, :cw], num[:, i, :cw])
                     for i, (c0, cw) in enumerate(slc)]
            views.append((1536, 32, dnt[:, 0, :], dnt[:, 1, :]))
            eng_f = nc.gpsimd if fin_mult == "pool" else nc.vector
            for c0, cw, dv, nv in views:
                sl = slice(c0, c0 + cw)
                nc.vector.reciprocal_approx_fast(out=rden[:, sl], in_=dv)
                eng_f.tensor_tensor(out=outsb[:, sl], in0=nv,
                                    in1=rden[:, sl],
                                    op=mybir.AluOpType.mult)
            nc.sync.dma_start(out=out, in_=outsb)

        cur_tiles, cur_items = make_p1(0)
        for it in cur_items:
            it()
        for r in range(reps):
            if r + 1 < reps:
                nxt_tiles, nxt_items = make_p1(r + 1)
            else:
                nxt_tiles, nxt_items = None, []
            phase2(cur_tiles, nxt_items)
            cur_tiles = nxt_tiles

    nc.finalize()
    return nc


def _prep_inputs(x, w_q, w_k, w_v, rel_h, rel_w):
    """Build the 8 per-core input dicts (all host-side numpy)."""
    import ml_dtypes
    x4 = np.ascontiguousarray(np.asarray(x, np.float32).reshape(B, H, W, CIN))
    relh = np.asarray(rel_h, np.float32).reshape(128, K)
    relw = np.asarray(rel_w, np.float32).reshape(128, K)
    ws = [np.asarray(w, np.float32) for w in (w_q, w_k, w_v)]
    ident = np.eye(128, dtype=ml_dtypes.bfloat16)
    nbias = np.full((128, 1), SHIFT, np.float32)

    in_maps = []
    for core in range(8):
        chalf, b, shalf = core >> 2, (core >> 1) & 1, core & 1
        if chalf == 0:
            xm = x4[b]
            rel = relh
        else:
            xm = x4[b].transpose(1, 0, 2)
            rel = relw
        arr = np.zeros((PR, 56, CIN), np.float32)
        if shalf == 0:
            arr[PAD:PAD + SPAN] = xm[0:SPAN]
        else:
            arr[0:SPAN] = xm[H - SPAN:H]
        xt = np.ascontiguousarray(
            arr.reshape(NPOS, CIN).T.astype(np.float16))
        cs = slice(chalf * 128, chalf * 128 + 128)
        wt = np.ascontiguousarray(
            np.stack([w[cs].T for w in ws]).astype(np.float16))
        in_maps.append({"xt": xt, "wt": wt, "rel": np.ascontiguousarray(rel),
                        "ident": ident, "nbias": nbias})
    return in_maps


def _make_runner(nc, n_cores=8):
    bass2jax.install_neuronx_cc_hook()
    in_names, out_names, out_avals = [], [], []
    partition_name = (nc.partition_id_tensor.name
                      if nc.partition_id_tensor else None)
    for alloc in nc.m.functions[0].allocations:
        if not isinstance(alloc, mybir.MemoryLocationSet):
            continue
        name = alloc.memorylocations[0].name
        if alloc.kind == "ExternalInput":
            if name != partition_name:
                in_names.append(name)
        elif alloc.kind == "ExternalOutput":
            out_names.append(name)
            shape = tuple(alloc.tensor_shape)
            dtype = mybir.dt.np(alloc.dtype)
            out_avals.append(jax.core.ShapedArray(shape, dtype))
    n_params = len(in_names)
    n_outs = len(out_names)
    all_names = list(in_names) + out_names
    if partition_name is not None:
        all_names.append(partition_name)

    def _body(*args):
        operands = list(args)
        if partition_name is not None:
            operands.append(bass2jax.partition_id_tensor())
        outs = bass2jax._bass_exec_p.bind(
            *operands, out_avals=tuple(out_avals), in_names=tuple(all_names),
            out_names=tuple(out_names), lowering_input_output_aliases=(),
            sim_require_finite=True, sim_require_nnan=True, nc=nc)
        return tuple(outs)

    devices = jax.devices()[:n_cores]
    mesh = Mesh(np.asarray(devices), ("core",))
    donate = tuple(range(n_params, n_params + n_outs))
    sharded = jax.jit(
        shard_map(_body, mesh=mesh,
                  in_specs=(PartitionSpec("core"),) * (n_params + n_outs),
                  out_specs=(PartitionSpec("core"),) * n_outs,
                  check_rep=False),
        donate_argnums=donate, keep_unused=True)
    return sharded, in_names, out_names, out_avals


def _get_compiled(reps=1, **kw):
    key = ("runner", reps, tuple(sorted(kw.items())))
    if key not in _CACHE:
        nc = _build_nc(reps=reps, **kw)
        _CACHE[key] = _make_runner(nc)
    return _CACHE[key]


def make_device_args(in_maps, reps=1, **kw):
    _, in_names, _, _ = _get_compiled(reps, **kw)
    return [np.concatenate([np.asarray(m[nm]) for m in in_maps], axis=0)
            for nm in in_names]


def run_cores(concat_in, reps=1, **kw):
    sharded, in_names, out_names, out_avals = _get_compiled(reps, **kw)
    concat_zeros = [np.zeros((8 * a.shape[0], *a.shape[1:]), a.dtype)
                    for a in out_avals]
    outs = sharded(*concat_in, *concat_zeros)
    o = np.asarray(outs[out_names.index("out")]).reshape(8, 128, NOWN)
    return o


def _assemble(per_core_out):
    out4 = np.empty((B, CO, H, W), np.float32)
    for core in range(8):
        chalf, b, shalf = core >> 2, (core >> 1) & 1, core & 1
        blk = per_core_out[core].reshape(128, OWN, 56)
        lo = shalf * OWN
        if chalf == 0:
            out4[b, 0:128, lo:lo + OWN, :] = blk
        else:
            out4[b, 128:256, :, lo:lo + OWN] = blk.transpose(0, 2, 1)
    return out4.reshape(B, CO * H, W)


def kernel(x, w_q, w_k, w_v, rel_h, rel_w):
    in_maps = _prep_inputs(x, w_q, w_k, w_v, rel_h, rel_w)
    concat_in = make_device_args(in_maps)
    per_core = run_cores(concat_in)
    return _assemble(per_core)


# revision 4
# speedup vs baseline: 1.1991x; 1.1991x over previous
"""AttentionConv (7x7 windowed per-channel softmax attention) on 8 TRN2 cores.

V2: d1-grouped mega-ops.  Sharding: core = (chalf, batch, shalf); chalf=1
maps stored transposed so rel_w folds like rel_h.  Per core: 128 channels
x 28x56 positions.

Per-rep structure (7 d1-iterations instead of 49 offset-iterations):
  phase 1 (PE, fp16 weights/x): q/k/v projections -> fp32 PSUM;
    k -> fp16 kpad, q -> fp16 qsb, v -> fp16 vpad (cast copies).
  per d1:
    kb = kpad rows + rel[:,d1]        one DVE tensor_scalar (4x fp16)
    s  = kb_view * q                  ONE DVE TT (2x fp16), 3-D in0 AP
                                      [d2:1, r:62, w:1], q bcast over d2
    e  = exp(s - 48) -> bf16          ONE ACT instr (10976 cols)
    t  = e * v_view -> bf16           ONE TT (DVE or Pool per T_POOL)
    den/num: per d2, per col-slice: PE identity matmuls accumulate into
      3x512 den + 3x512 num PSUM banks + shared 32-col tails (start=False
      on pre-zeroed sub-banks)
  out = num * reciprocal(den)         DVE
Cross-rep pipelining: next rep's phase-1 items drain between (d1,d2) MM
slices; PSUM layout: den 3 + num 3 + (tails+mm2) 1 + mm 1 = 8 banks, proj
groups double-buffered across mm/mm2.
"""
import numpy as np
from contextlib import ExitStack

import jax
from jax.sharding import Mesh, PartitionSpec
from jax.experimental.shard_map import shard_map

import concourse.bass as bass
import concourse.bacc as bacc
import concourse.tile as tile
from concourse import mybir
from concourse import bass2jax

F32 = mybir.dt.float32
BF16 = mybir.dt.bfloat16
FP16 = mybir.dt.float16

B, H, W, CIN, CO, K, PAD = 2, 56, 56, 512, 256, 7, 3
OWN = 28
SPAN = 31
PR = 34
PW = 62
NPOS = PR * 56      # 1904
NOWN = OWN * 56     # 1568
SHIFT = -48.0
NSL = 4
SLW = NOWN // NSL   # 392
ND1 = NOWN * K      # 10976 cols per d1 group

_CACHE = {}
# per d1 group, the t-mult for d2 in [0, t_pool[d1]) runs on GpSimd(Pool),
# the rest on DVE
T_POOL = (0, 0, 0, 0, 0, 0, 0)
BUFS = 2
NCH_DMA = 8
V_COPY = "dve"      # engine for v PSUM->SBUF cast copies
K_COPY = "act"      # engine for k PSUM->SBUF cast copies
FIN_MULT = "dve"    # engine for the final num*recip multiplies
GEXP = 2            # exp instructions per d1 group


KB_ENG = "dve"      # engine for the kb = kpad + rel folds


def _build_nc(reps=1, t_pool=T_POOL, bufs=BUFS, nch=NCH_DMA,
              v_copy=V_COPY, k_copy=K_COPY, fin_mult=FIN_MULT, gexp=GEXP,
              kb_eng=KB_ENG):
    nc = bacc.Bacc("TRN2", target_bir_lowering=False, debug=False)
    xt = nc.dram_tensor("xt", [CIN, NPOS], FP16, kind="ExternalInput").ap()
    wt = nc.dram_tensor("wt", [3, CIN, 128], FP16, kind="ExternalInput").ap()
    rel = nc.dram_tensor("rel", [128, K], F32, kind="ExternalInput").ap()
    ident = nc.dram_tensor("ident", [128, 128], BF16, kind="ExternalInput").ap()
    nbias = nc.dram_tensor("nbias", [128, 1], F32, kind="ExternalInput").ap()
    out = nc.dram_tensor("out", [128, NOWN], F32, kind="ExternalOutput").ap()

    # den/num col splits: 3x512 + 32-col tails in the shared dnt bank
    slc = [(0, 512), (512, 512), (1024, 512)]

    with tile.TileContext(nc) as tc, ExitStack() as ctx:
        per = ctx.enter_context(tc.tile_pool(name="per", bufs=1))
        ld = ctx.enter_context(tc.tile_pool(name="ld", bufs=1))

        wsb = ld.tile([128, 3, 4, 128], FP16)
        wtv = wt.rearrange("w (t p) m -> p w t m", p=128)
        nc.sync.dma_start(out=wsb[:, 1], in_=wtv[:, 1])   # k weights
        nc.sync.dma_start(out=wsb[:, 0], in_=wtv[:, 0])   # q weights
        relsb = per.tile([128, K], F32)
        nc.sync.dma_start(out=relsb, in_=rel)
        identsb = per.tile([128, 128], BF16)
        nc.sync.dma_start(out=identsb, in_=ident)
        nbsb = per.tile([128, 1], F32)
        nc.sync.dma_start(out=nbsb, in_=nbias)
        xsb = ld.tile([128, 4, NPOS], FP16)
        xtv = xt.rearrange("(t p) n -> p t n", p=128)
        chw = NPOS // nch
        for c in range(nch):
            for t in range(4):
                nc.sync.dma_start(out=xsb[:, t, c * chw:(c + 1) * chw],
                                  in_=xtv[:, t, c * chw:(c + 1) * chw])
        nc.sync.dma_start(out=wsb[:, 2], in_=wtv[:, 2])   # v weights

        maps = ctx.enter_context(tc.tile_pool(name="maps", bufs=2))
        kbp = ctx.enter_context(tc.tile_pool(name="kbp", bufs=3))
        sp = ctx.enter_context(tc.tile_pool(name="sp", bufs=bufs))
        ep = ctx.enter_context(tc.tile_pool(name="ep", bufs=bufs))
        tp = ctx.enter_context(tc.tile_pool(name="tp", bufs=bufs))
        fin = ctx.enter_context(tc.tile_pool(name="fin", bufs=2))
        # static PSUM layout (8 banks): mm 1 + den 3 + num 3 + (dnt+mm2) 1
        mm = ctx.enter_context(tc.tile_pool(name="mm", bufs=1, space="PSUM"))
        accp = ctx.enter_context(tc.tile_pool(name="acc", bufs=1,
                                              space="PSUM"))

        kv_slices = [(0, 7), (7, 7), (14, 7), (21, 7), (28, 6)]

        def make_p1(ri):
            """Allocate rep ri's map tiles and return (tiles, emit-closures)."""
            kpad = maps.tile([128, PR, PW], FP16, tag="kpad", name=f"kpad{ri}")
            vpad = maps.tile([128, PR, PW], FP16, tag="vpad", name=f"vpad{ri}")
            qsb = maps.tile([128, NOWN], FP16, tag="qsb", name=f"qsb{ri}")
            tiles = (kpad, vpad, qsb)

            def memsets():
                for buf in (kpad, vpad):
                    nc.gpsimd.memset(buf[:, :, 0:PAD], 0.0)
                    nc.gpsimd.memset(buf[:, :, PAD + 56:PW], 0.0)

            def proj_kv(wi, dst, r0, nr, eng_copy):
                pt = mm.tile([128, 392], F32, tag="mmkv", name="pt")
                ptv = pt[:, :nr * 56]
                n0, n1 = r0 * 56, (r0 + nr) * 56
                for t in range(4):
                    nc.tensor.matmul(ptv,
                                     lhsT=wsb[:, wi, t, :],
                                     rhs=xsb[:, t, n0:n1],
                                     start=(t == 0), stop=(t == 3))
                dstv = dst[:, r0:r0 + nr, PAD:PAD + 56]
                srcv = ptv.rearrange("p (r c) -> p r c", r=nr)
                if eng_copy == "act":
                    nc.scalar.copy(out=dstv, in_=srcv)
                elif eng_copy == "pool":
                    nc.gpsimd.tensor_copy(out=dstv, in_=srcv)
                else:
                    nc.vector.tensor_copy(out=dstv, in_=srcv)

            def proj_q(i):
                pt = mm.tile([128, 392], F32, tag="mmkv", name="pt")
                n0 = PAD * 56 + i * SLW
                for t in range(4):
                    nc.tensor.matmul(pt, lhsT=wsb[:, 0, t, :],
                                     rhs=xsb[:, t, n0:n0 + SLW],
                                     start=(t == 0), stop=(t == 3))
                nc.scalar.copy(out=qsb[:, i * SLW:(i + 1) * SLW], in_=pt)

            from functools import partial
            items = [memsets]
            items += [partial(proj_kv, 1, kpad, r0, nr, k_copy)
                      for (r0, nr) in kv_slices]
            items += [partial(proj_q, i) for i in range(NSL)]
            items += [partial(proj_kv, 2, vpad, r0, nr, v_copy)
                      for (r0, nr) in kv_slices]
            return tiles, items

        def phase2(tiles, nxt_items):
            """Emit rep's attention stream, draining nxt_items between MM
            slices."""
            kpad, vpad, qsb = tiles
            den = accp.tile([128, 3, 512], F32, tag="den", name="den")
            num = accp.tile([128, 3, 512], F32, tag="num", name="num")
            dnt = accp.tile([128, 2, 32], F32, tag="dnt", name="dnt")
            nc.vector.memset(dnt, 0.0)

            q3 = qsb.rearrange("p (r c) -> p r c", r=OWN)
            nit = len(nxt_items)
            drained = 0
            j = 0
            NSLOT = K * K
            for d1 in range(K):
                kb = kbp.tile([128, OWN, PW], FP16, tag="kb", name=f"kb{d1}")
                if kb_eng == "act":
                    nc.scalar.activation(
                        out=kb, in_=kpad[:, d1:d1 + OWN, :],
                        func=mybir.ActivationFunctionType.Identity,
                        bias=relsb[:, d1:d1 + 1], scale=1.0)
                elif kb_eng == "pool":
                    nc.gpsimd.tensor_scalar_add(out=kb,
                                                in0=kpad[:, d1:d1 + OWN, :],
                                                scalar1=relsb[:, d1:d1 + 1])
                else:
                    nc.vector.tensor_scalar_add(out=kb,
                                                in0=kpad[:, d1:d1 + OWN, :],
                                                scalar1=relsb[:, d1:d1 + 1])
                st = sp.tile([128, K, OWN, 56], FP16, tag="s", name="st")
                # in0[d2, r, w] = kb[r, d2 + w]; q broadcast over d2
                kbw = bass.AP(tensor=kb.tensor, offset=kb.offset,
                              ap=mybir.VecI64Pair(
                                  [list(kb.ap[0]), [1, K], [PW, OWN],
                                   [1, 56]]))
                qb = bass.AP(tensor=q3.tensor, offset=q3.offset,
                             ap=mybir.VecI64Pair(
                                 [list(q3.ap[0]), [0, K], [56, OWN],
                                  [1, 56]]))
                nc.vector.tensor_tensor(
                    out=st,
                    in0=kbw,
                    in1=qb,
                    op=mybir.AluOpType.mult)
                et = ep.tile([128, K, OWN, 56], BF16, tag="e", name="et")
                esplits = ([(0, K)] if gexp == 1 else
                           [(i * K // gexp, (i + 1) * K // gexp)
                            for i in range(gexp)])
                for lo, hi in esplits:
                    nc.scalar.activation(
                        out=et[:, lo:hi], in_=st[:, lo:hi],
                        func=mybir.ActivationFunctionType.Exp,
                        bias=nbsb, scale=1.0)
                tt = tp.tile([128, K, OWN, 56], BF16, tag="t", name="tt")

                def t_part(eng, lo, hi):
                    vw = bass.AP(
                        tensor=vpad.tensor,
                        offset=vpad.offset + d1 * PW + lo,
                        ap=mybir.VecI64Pair(
                            [list(vpad.ap[0]), [1, hi - lo], [PW, OWN],
                             [1, 56]]))
                    eng.tensor_tensor(out=tt[:, lo:hi], in0=et[:, lo:hi],
                                      in1=vw, op=mybir.AluOpType.mult)

                np_ = t_pool[d1]
                for i in range(np_):           # single-d2 Pool ops
                    t_part(nc.gpsimd, i, i + 1)
                if np_ < K:
                    t_part(nc.vector, np_, K)

                e2 = et.rearrange("p k r c -> p (k r c)")
                t2 = tt.rearrange("p k r c -> p (k r c)")
                for d2 in range(K):
                    want = min(nit, nit * (j + 1) // NSLOT + 1)
                    while drained < want:
                        nxt_items[drained]()
                        drained += 1
                    first = (d1 == 0 and d2 == 0)
                    last = (d1 == K - 1 and d2 == K - 1)
                    o2 = d2 * NOWN
                    for i, (c0, cw) in enumerate(slc):
                        nc.tensor.matmul(
                            den[:, i, :cw], lhsT=identsb,
                            rhs=e2[:, o2 + c0:o2 + c0 + cw],
                            start=first, stop=last, skip_group_check=True)
                        nc.tensor.matmul(
                            num[:, i, :cw], lhsT=identsb,
                            rhs=t2[:, o2 + c0:o2 + c0 + cw],
                            start=first, stop=last, skip_group_check=True)
                    nc.tensor.matmul(
                        dnt[:, 0, :], lhsT=identsb,
                        rhs=e2[:, o2 + 1536:o2 + NOWN],
                        start=False, stop=last, skip_group_check=True)
                    nc.tensor.matmul(
                        dnt[:, 1, :], lhsT=identsb,
                        rhs=t2[:, o2 + 1536:o2 + NOWN],
                        start=False, stop=last, skip_group_check=True)
                    j += 1
            while drained < nit:
                nxt_items[drained]()
                drained += 1

            rden = fin.tile([128, NOWN], F32, tag="rden", name="rden")
            outsb = fin.tile([128, NOWN], F32, tag="outsb", name="outsb")
            views = [(c0, cw, den[:, i, :cw], num[:, i, :cw])
                     for i, (c0, cw) in enumerate(slc)]
            views.append((1536, 32, dnt[:, 0, :], dnt[:, 1, :]))
            eng_f = nc.gpsimd if fin_mult == "pool" else nc.vector
            for c0, cw, dv, nv in views:
                sl = slice(c0, c0 + cw)
                nc.vector.reciprocal_approx_fast(out=rden[:, sl], in_=dv)
                eng_f.tensor_tensor(out=outsb[:, sl], in0=nv,
                                    in1=rden[:, sl],
                                    op=mybir.AluOpType.mult)
            nc.sync.dma_start(out=out, in_=outsb)

        cur_tiles, cur_items = make_p1(0)
        for it in cur_items:
            it()
        for r in range(reps):
            if r + 1 < reps:
                nxt_tiles, nxt_items = make_p1(r + 1)
            else:
                nxt_tiles, nxt_items = None, []
            phase2(cur_tiles, nxt_items)
            cur_tiles = nxt_tiles

    nc.finalize()
    return nc


def _prep_inputs(x, w_q, w_k, w_v, rel_h, rel_w):
    """Build the 8 per-core input dicts (all host-side numpy)."""
    import ml_dtypes
    x4 = np.ascontiguousarray(np.asarray(x, np.float32).reshape(B, H, W, CIN))
    relh = np.asarray(rel_h, np.float32).reshape(128, K)
    relw = np.asarray(rel_w, np.float32).reshape(128, K)
    ws = [np.asarray(w, np.float32) for w in (w_q, w_k, w_v)]
    ident = np.eye(128, dtype=ml_dtypes.bfloat16)
    nbias = np.full((128, 1), SHIFT, np.float32)

    in_maps = []
    for core in range(8):
        chalf, b, shalf = core >> 2, (core >> 1) & 1, core & 1
        if chalf == 0:
            xm = x4[b]
            rel = relh
        else:
            xm = x4[b].transpose(1, 0, 2)
            rel = relw
        arr = np.zeros((PR, 56, CIN), np.float32)
        if shalf == 0:
            arr[PAD:PAD + SPAN] = xm[0:SPAN]
        else:
            arr[0:SPAN] = xm[H - SPAN:H]
        xt = np.ascontiguousarray(
            arr.reshape(NPOS, CIN).T.astype(np.float16))
        cs = slice(chalf * 128, chalf * 128 + 128)
        wt = np.ascontiguousarray(
            np.stack([w[cs].T for w in ws]).astype(np.float16))
        in_maps.append({"xt": xt, "wt": wt, "rel": np.ascontiguousarray(rel),
                        "ident": ident, "nbias": nbias})
    return in_maps


def _make_runner(nc, n_cores=8):
    bass2jax.install_neuronx_cc_hook()
    in_names, out_names, out_avals = [], [], []
    partition_name = (nc.partition_id_tensor.name
                      if nc.partition_id_tensor else None)
    for alloc in nc.m.functions[0].allocations:
        if not isinstance(alloc, mybir.MemoryLocationSet):
            continue
        name = alloc.memorylocations[0].name
        if alloc.kind == "ExternalInput":
            if name != partition_name:
                in_names.append(name)
        elif alloc.kind == "ExternalOutput":
            out_names.append(name)
            shape = tuple(alloc.tensor_shape)
            dtype = mybir.dt.np(alloc.dtype)
            out_avals.append(jax.core.ShapedArray(shape, dtype))
    n_params = len(in_names)
    n_outs = len(out_names)
    all_names = list(in_names) + out_names
    if partition_name is not None:
        all_names.append(partition_name)

    def _body(*args):
        operands = list(args)
        if partition_name is not None:
            operands.append(bass2jax.partition_id_tensor())
        outs = bass2jax._bass_exec_p.bind(
            *operands, out_avals=tuple(out_avals), in_names=tuple(all_names),
            out_names=tuple(out_names), lowering_input_output_aliases=(),
            sim_require_finite=True, sim_require_nnan=True, nc=nc)
        return tuple(outs)

    devices = jax.devices()[:n_cores]
    mesh = Mesh(np.asarray(devices), ("core",))
    donate = tuple(range(n_params, n_params + n_outs))
    sharded = jax.jit(
        shard_map(_body, mesh=mesh,
                  in_specs=(PartitionSpec("core"),) * (n_params + n_outs),
                  out_specs=(PartitionSpec("core"),) * n_outs,
                  check_rep=False),
        donate_argnums=donate, keep_unused=True)
    return sharded, in_names, out_names, out_avals


def _get_compiled(reps=1, **kw):
    key = ("runner", reps, tuple(sorted(kw.items())))
    if key not in _CACHE:
        nc = _build_nc(reps=reps, **kw)
        _CACHE[key] = _make_runner(nc)
    return _CACHE[key]


def make_device_args(in_maps, reps=1, **kw):
    _, in_names, _, _ = _get_compiled(reps, **kw)
    return [np.concatenate([np.asarray(m[nm]) for m in in_maps], axis=0)
            for nm in in_names]


def run_cores(concat_in, reps=1, **kw):
    sharded, in_names, out_names, out_avals = _get_compiled(reps, **kw)
    concat_zeros = [np.zeros((8 * a.shape[0], *a.shape[1:]), a.dtype)
                    for a in out_avals]
    outs = sharded(*concat_in, *concat_zeros)
    o = np.asarray(outs[out_names.index("out")]).reshape(8, 128, NOWN)
    return o


def _assemble(per_core_out):
    out4 = np.empty((B, CO, H, W), np.float32)
    for core in range(8):
        chalf, b, shalf = core >> 2, (core >> 1) & 1, core & 1
        blk = per_core_out[core].reshape(128, OWN, 56)
        lo = shalf * OWN
        if chalf == 0:
            out4[b, 0:128, lo:lo + OWN, :] = blk
        else:
            out4[b, 128:256, :, lo:lo + OWN] = blk.transpose(0, 2, 1)
    return out4.reshape(B, CO * H, W)


def kernel(x, w_q, w_k, w_v, rel_h, rel_w):
    in_maps = _prep_inputs(x, w_q, w_k, w_v, rel_h, rel_w)
    concat_in = make_device_args(in_maps)
    per_core = run_cores(concat_in)
    return _assemble(per_core)
